# revision 3
# baseline (speedup 1.0000x reference)
"""DualGraphSHM kernel: conv1+conv2+F2 on 8 trn2 cores (bass/tile), rest on host.

Device per core (128 samples): x -> conv1 -> relu -> conv2 -> relu ->
xbar-transpose -> F2 = relu(h2 @ Wt + bt).  Layout: 4 samples stacked on
partitions at 32-row pitch ("slots"), time on the free dim with 2-col zero
padding per sample block so SAME-conv offsets never cross samples.
"""
import numpy as np
import ml_dtypes

import concourse.bacc as bacc
import concourse.mybir as mybir
import concourse.tile as tile
from concourse import bass_utils

BF = ml_dtypes.bfloat16
NCORES = 8
B, S, T, FD, NCLS = 1024, 30, 1024, 200, 7
BC = B // NCORES          # 128 samples per core
G = BC // 4               # 32 groups of 4 stacked samples
GH = G // 2               # 16 groups per half (SBUF pressure)
PITCH = 1028              # per-group col pitch: [2 zero][1024 data][2 zero]

_noisy = False


def _build_device_program():
    nc = bacc.Bacc("TRN2", target_bir_lowering=False, debug=False,
                   num_devices=NCORES)
    dt = mybir.dt
    xs = nc.dram_tensor("xs", [128, G * PITCH], dt.bfloat16,
                        kind="ExternalInput").ap()
    w1 = nc.dram_tensor("w1", [128, 3 * 128], dt.bfloat16,
                        kind="ExternalInput").ap()
    w2 = nc.dram_tensor("w2", [128, 3 * 128], dt.bfloat16,
                        kind="ExternalInput").ap()
    wt = nc.dram_tensor("wt", [128, 8 * FD], dt.bfloat16,
                        kind="ExternalInput").ap()
    b1 = nc.dram_tensor("b1", [128, 1], dt.float32, kind="ExternalInput").ap()
    b2 = nc.dram_tensor("b2", [128, 1], dt.float32, kind="ExternalInput").ap()
    bt = nc.dram_tensor("bt", [128, 2], dt.float32, kind="ExternalInput").ap()
    f2o = nc.dram_tensor("f2o", [FD, BC * 32], dt.float32,
                         kind="ExternalOutput").ap()

    with tile.TileContext(nc) as tc:
        with (
            tc.tile_pool(name="consts", bufs=1) as cpool,
            tc.tile_pool(name="big", bufs=1) as bpool,
            tc.tile_pool(name="ps", bufs=4, space="PSUM") as pspool,
            tc.tile_pool(name="f2ps", bufs=2, space="PSUM") as f2pspool,
            tc.tile_pool(name="ev", bufs=2) as evpool,
        ):
            w1s = cpool.tile([128, 3 * 128], dt.bfloat16, tag="w1")
            w2s = cpool.tile([128, 3 * 128], dt.bfloat16, tag="w2")
            wts = cpool.tile([128, 8 * FD], dt.bfloat16, tag="wt")
            b1s = cpool.tile([128, 1], dt.float32, tag="b1")
            b2s = cpool.tile([128, 1], dt.float32, tag="b2")
            bts = cpool.tile([128, 2], dt.float32, tag="bt")
            nc.sync.dma_start(w1s[:], w1[:])
            nc.sync.dma_start(w2s[:], w2[:])
            nc.sync.dma_start(wts[:], wt[:])
            nc.sync.dma_start(b1s[:], b1[:])
            nc.sync.dma_start(b2s[:], b2[:])
            nc.sync.dma_start(bts[:], bt[:])

            relu = mybir.ActivationFunctionType.Relu

            for half in range(2):
                g0 = half * GH
                xsh = bpool.tile([128, GH * PITCH], dt.bfloat16, tag="xsh")
                nc.sync.dma_start(
                    xsh[:], xs[:, g0 * PITCH:(g0 + GH) * PITCH])
                h1 = bpool.tile([128, GH * PITCH], dt.bfloat16, tag="h1")
                # zero the 4 pad cols of each block (conv2 reads them)
                nc.vector.memset(
                    h1[:].rearrange("p (g c) -> p g c", c=PITCH)[:, :, 0:2], 0.0)
                nc.vector.memset(
                    h1[:].rearrange("p (g c) -> p g c", c=PITCH)[:, :, 1026:1028],
                    0.0)
                # ---- conv1 ----
                for g in range(GH):
                    base = g * PITCH
                    for hlf in range(2):
                        ps = pspool.tile([128, 512], dt.float32, tag="cps")
                        for k in range(3):
                            nc.tensor.matmul(
                                ps[:],
                                w1s[:, k * 128:(k + 1) * 128],
                                xsh[:, base + 1 + hlf * 512 + k:
                                    base + 1 + hlf * 512 + k + 512],
                                start=(k == 0), stop=(k == 2))
                        nc.scalar.activation(
                            h1[:, base + 2 + hlf * 512:base + 2 + hlf * 512 + 512],
                            ps[:], relu, bias=b1s[:])
                # ---- conv2 ----
                h2 = bpool.tile([128, GH * 1024], dt.bfloat16, tag="h2")
                for g in range(GH):
                    base = g * PITCH
                    for hlf in range(2):
                        ps = pspool.tile([128, 512], dt.float32, tag="cps")
                        for k in range(3):
                            nc.tensor.matmul(
                                ps[:],
                                w2s[:, k * 128:(k + 1) * 128],
                                h1[:, base + 1 + hlf * 512 + k:
                                   base + 1 + hlf * 512 + k + 512],
                                start=(k == 0), stop=(k == 2))
                        nc.scalar.activation(
                            h2[:, g * 1024 + hlf * 512:g * 1024 + hlf * 512 + 512],
                            ps[:], relu, bias=b2s[:])
                # ---- transpose h2: per group [128, 1024] -> [128, 8, 128] ----
                h2t = bpool.tile([128, GH * 1024], dt.bfloat16, tag="h2t")
                for g in range(GH):
                    nc.sync.dma_start(
                        h2t[:, g * 1024:(g + 1) * 1024].rearrange(
                            "p (c f) -> p c f", f=128),
                        h2[:, g * 1024:(g + 1) * 1024],
                        transpose=True)
                # ---- F2^T = relu(Wt^T @ h2t + bt) ----
                for sub in range(4):          # 4 groups per matmul -> N=512
                    for ft in range(2):       # f tiles: 128 + 72
                        fw = 128 if ft == 0 else FD - 128
                        ps = f2pspool.tile([128, 512], dt.float32, tag="f2ps")
                        for tt in range(8):
                            nc.tensor.matmul(
                                ps[:fw, :],
                                wts[:, tt * FD + ft * 128:tt * FD + ft * 128 + fw],
                                h2t[:, :].rearrange(
                                    "p (g c s) -> p g c s", c=8, s=128)[
                                    :, sub * 4:(sub + 1) * 4, tt, :],
                                start=(tt == 0), stop=(tt == 7))
                        ev = evpool.tile([128, 512], dt.float32, tag="f2ev")
                        nc.scalar.activation(ev[:fw, :], ps[:fw, :], relu,
                                             bias=bts[:fw, ft:ft + 1])
                        nc.sync.dma_start(
                            f2o[ft * 128:ft * 128 + fw,
                                half * 2048 + sub * 512:
                                half * 2048 + sub * 512 + 512],
                            ev[:fw, :])
    nc.compile()
    return nc


_nc_cache = None


def _get_nc():
    global _nc_cache
    if _nc_cache is None:
        _nc_cache = _build_device_program()
    return _nc_cache


def _host_inputs(x, Wc1, bc1, Wc2, bc2, Wt, bt):
    """Build per-core input dicts."""
    xf = np.asarray(x, np.float32)
    ins = []
    # block-diag conv weights: lhsT[32a+i, 32a+o] = W[o, i, k]
    def bdw(W):
        out = np.zeros((128, 3 * 128), np.float32)
        for k in range(3):
            for a in range(4):
                out[32 * a:32 * a + S, k * 128 + 32 * a:k * 128 + 32 * a + S] = \
                    np.asarray(W, np.float32)[:, :, k].T
        return out.astype(BF)
    w1h, w2h = bdw(Wc1), bdw(Wc2)
    wth = np.zeros((128, 8 * FD), np.float32)
    wtf = np.asarray(Wt, np.float32)
    for tt in range(8):
        wth[:, tt * FD:(tt + 1) * FD] = wtf[tt * 128:(tt + 1) * 128, :]
    wth = wth.astype(BF)
    bstack = np.zeros((128, 1), np.float32)
    b1h = bstack.copy(); b2h = bstack.copy()
    for a in range(4):
        b1h[32 * a:32 * a + S, 0] = np.asarray(bc1, np.float32)
        b2h[32 * a:32 * a + S, 0] = np.asarray(bc2, np.float32)
    bth = np.zeros((128, 2), np.float32)
    btf = np.asarray(bt, np.float32)
    bth[:, 0] = btf[:128]
    bth[:72, 1] = btf[128:]
    for c in range(NCORES):
        xc = xf[c * BC:(c + 1) * BC]            # [128, 30, 1024]
        xsa = np.zeros((128, G, PITCH), np.float32)
        xg = xc.reshape(G, 4, S, T)             # sample = 4*g + a
        for a in range(4):
            xsa[32 * a:32 * a + S, :, 2:2 + T] = xg[:, a].transpose(1, 0, 2)
        ins.append(dict(
            xs=np.ascontiguousarray(xsa.reshape(128, G * PITCH)).astype(BF),
            w1=w1h, w2=w2h, wt=wth, b1=b1h, b2=b2h, bt=bth))
    return ins


def _host_post(F2, adj_self, Wa, Wm1, Wm2, Wm3, Wg1, Wg2, wg,
               Wp1, Wp2, Wp3, Wl, Wgl, Ws1, Ws2, Wf1, Wf2, Wcls, bcls):
    """Numpy port of reference() from F2 onward. F2: [B, S, FD] float32."""
    A = np.asarray(adj_self, np.float32)
    f = lambda w: np.asarray(w, np.float32)
    relu = lambda v: np.maximum(v, 0.0)
    P = F2 @ f(Wa)                                   # [B,S,FD]
    M = np.einsum('big,bjg->bij', P, F2)
    Mr = relu(M)
    E = np.exp(Mr - Mr.max(-1, keepdims=True))
    A_F = E / E.sum(-1, keepdims=True)
    gc = lambda Am, X, W: relu(np.einsum('bij,bjf->bif', Am, X) @ W) \
        if Am.ndim == 3 else relu(np.einsum('ij,bjf->bif', Am, X) @ W)
    x1 = gc(A_F, F2, f(Wm1))
    x2 = gc(A_F, x1, f(Wm2))
    x3 = gc(A_F, x2, f(Wm3))
    h1 = relu(np.einsum('ij,bjf->bif', A, F2) @ f(Wg1))
    xs = np.einsum('ij,bjf->bif', A, h1) @ f(Wg2)
    H1, H2, H3 = (x1 + xs) * .5, (x2 + xs) * .5, (x3 + xs) * .5
    wgf = f(wg)
    sc = np.stack([H @ wgf[:, k] for k, H in enumerate((H1, H2, H3))], -1)
    e = np.exp(sc - sc.max(-1, keepdims=True))
    g = e / e.sum(-1, keepdims=True)
    agg = lambda k, H, Wp: np.einsum('ij,bjf->bif',
                                     A, g[..., k:k + 1] * H) @ f(Wp)
    G_h = np.concatenate([agg(0, H1, Wp1), agg(1, H2, Wp2),
                          agg(2, H3, Wp3)], -1)
    loc = relu(np.einsum('ij,bjf->bif', A, F2) @ f(Wl))
    glb = relu(np.einsum('bij,bjf->bif', A_F, F2) @ f(Wgl))
    G_v = np.concatenate([loc, glb], -1)
    sig = lambda v: 1.0 / (1.0 + np.exp(-v))
    wch = sig(relu(G_v.mean(-1) @ f(Ws1)) @ f(Ws2))
    G_h_att = G_h * wch[:, :, None]
    wft = sig(relu(G_h.mean(1) @ f(Wf1)) @ f(Wf2))
    G_v_att = G_v * wft[:, None, :]
    Gc = np.concatenate([G_h_att, G_v_att], -1).reshape(F2.shape[0], -1)
    logits = Gc @ f(Wcls) + f(bcls)
    lse = logits - logits.max(-1, keepdims=True)
    return (lse - np.log(np.exp(lse).sum(-1, keepdims=True))).astype(np.float32)


def kernel(x, adj_self, Wc1, bc1, Wc2, bc2, Wt, bt, Wa, Wm1, Wm2, Wm3,
           Wg1, Wg2, wg, Wp1, Wp2, Wp3, Wl, Wgl, Ws1, Ws2, Wf1, Wf2,
           Wcls, bcls, _trace=False):
    nc = _get_nc()
    ins = _host_inputs(x, Wc1, bc1, Wc2, bc2, Wt, bt)
    res = bass_utils.run_bass_kernel_spmd(
        nc, ins, core_ids=list(range(NCORES)), trace=_trace)
    F2 = np.empty((B, S, FD), np.float32)
    for c in range(NCORES):
        o = np.asarray(res.results[c]["f2o"], np.float32)  # [FD, 4096]
        o4 = o.reshape(FD, G, 4, 32)                        # f, group, a, slot
        F2[c * BC:(c + 1) * BC] = o4[:, :, :, :S].transpose(1, 2, 3, 0).reshape(
            BC, S, FD)
    out = _host_post(F2, adj_self, Wa, Wm1, Wm2, Wm3, Wg1, Wg2, wg,
                     Wp1, Wp2, Wp3, Wl, Wgl, Ws1, Ws2, Wf1, Wf2, Wcls, bcls)
    if _trace:
        kernel.last_exec_time_ns = res.exec_time_ns
    return out


# revision 4
# speedup vs baseline: 1.0040x; 1.0040x over previous
"""DualGraphSHM kernel: conv1+conv2+F2 on 8 trn2 cores (bass/tile), rest on host.

Device per core (128 samples): x -> conv1 -> relu -> conv2 -> relu ->
xbar-transpose -> F2 = relu(h2 @ Wt + bt).  Layout: 4 samples stacked on
partitions at 32-row pitch ("slots"), time on the free dim with 2-col zero
padding per sample block so SAME-conv offsets never cross samples.
"""
import numpy as np
import ml_dtypes

import concourse.bacc as bacc
import concourse.mybir as mybir
import concourse.tile as tile
from concourse import bass_utils

BF = ml_dtypes.bfloat16
NCORES = 8
B, S, T, FD, NCLS = 1024, 30, 1024, 200, 7
BC = B // NCORES          # 128 samples per core
G = BC // 4               # 32 groups of 4 stacked samples
GH = G // 2               # 16 groups per half (SBUF pressure)
PITCH = 1028              # per-group col pitch: [2 zero][1024 data][2 zero]

_noisy = False


def _build_device_program():
    nc = bacc.Bacc("TRN2", target_bir_lowering=False, debug=False,
                   num_devices=NCORES)
    dt = mybir.dt
    xs = nc.dram_tensor("xs", [128, G * PITCH], dt.bfloat16,
                        kind="ExternalInput").ap()
    w1 = nc.dram_tensor("w1", [128, 3 * 128], dt.bfloat16,
                        kind="ExternalInput").ap()
    w2 = nc.dram_tensor("w2", [128, 3 * 128], dt.bfloat16,
                        kind="ExternalInput").ap()
    wt = nc.dram_tensor("wt", [128, 8 * FD], dt.bfloat16,
                        kind="ExternalInput").ap()
    b1 = nc.dram_tensor("b1", [128, 1], dt.float32, kind="ExternalInput").ap()
    b2 = nc.dram_tensor("b2", [128, 1], dt.float32, kind="ExternalInput").ap()
    bt = nc.dram_tensor("bt", [128, 2], dt.float32, kind="ExternalInput").ap()
    f2o = nc.dram_tensor("f2o", [FD, BC * 32], dt.float32,
                         kind="ExternalOutput").ap()

    with tile.TileContext(nc) as tc:
        with (
            tc.tile_pool(name="consts", bufs=1) as cpool,
            tc.tile_pool(name="big", bufs=1) as bpool,
            tc.tile_pool(name="ps", bufs=6, space="PSUM") as pspool,
            tc.tile_pool(name="f2ps", bufs=2, space="PSUM") as f2pspool,
            tc.tile_pool(name="ev", bufs=2) as evpool,
        ):
            w1s = cpool.tile([128, 3 * 128], dt.bfloat16, tag="w1")
            w2s = cpool.tile([128, 3 * 128], dt.bfloat16, tag="w2")
            wts = cpool.tile([128, 8 * FD], dt.bfloat16, tag="wt")
            b1s = cpool.tile([128, 1], dt.float32, tag="b1")
            b2s = cpool.tile([128, 1], dt.float32, tag="b2")
            bts = cpool.tile([128, 2], dt.float32, tag="bt")
            nc.sync.dma_start(w1s[:], w1[:])
            nc.sync.dma_start(w2s[:], w2[:])
            nc.sync.dma_start(wts[:], wt[:])
            nc.sync.dma_start(b1s[:], b1[:])
            nc.sync.dma_start(b2s[:], b2[:])
            nc.sync.dma_start(bts[:], bt[:])

            relu = mybir.ActivationFunctionType.Relu

            for half in range(2):
                g0 = half * GH
                xsh = bpool.tile([128, GH * PITCH], dt.bfloat16, tag="xsh")
                nc.sync.dma_start(
                    xsh[:], xs[:, g0 * PITCH:(g0 + GH) * PITCH])
                h1 = bpool.tile([128, GH * PITCH], dt.bfloat16, tag="h1")
                # zero the 4 pad cols of each block (conv2 reads them)
                nc.vector.memset(
                    h1[:].rearrange("p (g c) -> p g c", c=PITCH)[:, :, 0:2], 0.0)
                nc.vector.memset(
                    h1[:].rearrange("p (g c) -> p g c", c=PITCH)[:, :, 1026:1028],
                    0.0)
                # ---- conv1 ----
                for g in range(GH):
                    base = g * PITCH
                    for hlf in range(2):
                        ps = pspool.tile([128, 512], dt.float32, tag="cps")
                        for k in range(3):
                            nc.tensor.matmul(
                                ps[:],
                                w1s[:, k * 128:(k + 1) * 128],
                                xsh[:, base + 1 + hlf * 512 + k:
                                    base + 1 + hlf * 512 + k + 512],
                                start=(k == 0), stop=(k == 2))
                        if hlf == 0:
                            nc.scalar.activation(
                                h1[:, base + 2:base + 2 + 512],
                                ps[:], relu, bias=b1s[:])
                        else:
                            nc.vector.tensor_scalar(
                                h1[:, base + 2 + 512:base + 2 + 1024],
                                ps[:], b1s[:], 0.0,
                                mybir.AluOpType.add, mybir.AluOpType.max)
                # ---- conv2 ----
                h2 = bpool.tile([128, GH * 1024], dt.bfloat16, tag="h2")
                for g in range(GH):
                    base = g * PITCH
                    for hlf in range(2):
                        ps = pspool.tile([128, 512], dt.float32, tag="cps")
                        for k in range(3):
                            nc.tensor.matmul(
                                ps[:],
                                w2s[:, k * 128:(k + 1) * 128],
                                h1[:, base + 1 + hlf * 512 + k:
                                   base + 1 + hlf * 512 + k + 512],
                                start=(k == 0), stop=(k == 2))
                        if hlf == 0:
                            nc.scalar.activation(
                                h2[:, g * 1024:g * 1024 + 512],
                                ps[:], relu, bias=b2s[:])
                        else:
                            nc.vector.tensor_scalar(
                                h2[:, g * 1024 + 512:g * 1024 + 1024],
                                ps[:], b2s[:], 0.0,
                                mybir.AluOpType.add, mybir.AluOpType.max)
                # ---- transpose h2: per group [128, 1024] -> [128, 8, 128] ----
                h2t = bpool.tile([128, GH * 1024], dt.bfloat16, tag="h2t")
                for g in range(GH):
                    nc.sync.dma_start(
                        h2t[:, g * 1024:(g + 1) * 1024].rearrange(
                            "p (c f) -> p c f", f=128),
                        h2[:, g * 1024:(g + 1) * 1024],
                        transpose=True)
                # ---- F2^T = relu(Wt^T @ h2t + bt) ----
                for sub in range(4):          # 4 groups per matmul -> N=512
                    for ft in range(2):       # f tiles: 128 + 72
                        fw = 128 if ft == 0 else FD - 128
                        ps = f2pspool.tile([128, 512], dt.float32, tag="f2ps")
                        for tt in range(8):
                            nc.tensor.matmul(
                                ps[:fw, :],
                                wts[:, tt * FD + ft * 128:tt * FD + ft * 128 + fw],
                                h2t[:, :].rearrange(
                                    "p (g c s) -> p g c s", c=8, s=128)[
                                    :, sub * 4:(sub + 1) * 4, tt, :],
                                start=(tt == 0), stop=(tt == 7))
                        ev = evpool.tile([128, 512], dt.float32, tag="f2ev")
                        nc.scalar.activation(ev[:fw, :], ps[:fw, :], relu,
                                             bias=bts[:fw, ft:ft + 1])
                        nc.sync.dma_start(
                            f2o[ft * 128:ft * 128 + fw,
                                half * 2048 + sub * 512:
                                half * 2048 + sub * 512 + 512],
                            ev[:fw, :])
    nc.compile()
    return nc


_nc_cache = None


def _get_nc():
    global _nc_cache
    if _nc_cache is None:
        _nc_cache = _build_device_program()
    return _nc_cache


def _host_inputs(x, Wc1, bc1, Wc2, bc2, Wt, bt):
    """Build per-core input dicts."""
    xf = np.asarray(x, np.float32)
    ins = []
    # block-diag conv weights: lhsT[32a+i, 32a+o] = W[o, i, k]
    def bdw(W):
        out = np.zeros((128, 3 * 128), np.float32)
        for k in range(3):
            for a in range(4):
                out[32 * a:32 * a + S, k * 128 + 32 * a:k * 128 + 32 * a + S] = \
                    np.asarray(W, np.float32)[:, :, k].T
        return out.astype(BF)
    w1h, w2h = bdw(Wc1), bdw(Wc2)
    wth = np.zeros((128, 8 * FD), np.float32)
    wtf = np.asarray(Wt, np.float32)
    for tt in range(8):
        wth[:, tt * FD:(tt + 1) * FD] = wtf[tt * 128:(tt + 1) * 128, :]
    wth = wth.astype(BF)
    bstack = np.zeros((128, 1), np.float32)
    b1h = bstack.copy(); b2h = bstack.copy()
    for a in range(4):
        b1h[32 * a:32 * a + S, 0] = np.asarray(bc1, np.float32)
        b2h[32 * a:32 * a + S, 0] = np.asarray(bc2, np.float32)
    bth = np.zeros((128, 2), np.float32)
    btf = np.asarray(bt, np.float32)
    bth[:, 0] = btf[:128]
    bth[:72, 1] = btf[128:]
    for c in range(NCORES):
        xc = xf[c * BC:(c + 1) * BC]            # [128, 30, 1024]
        xsa = np.zeros((128, G, PITCH), np.float32)
        xg = xc.reshape(G, 4, S, T)             # sample = 4*g + a
        for a in range(4):
            xsa[32 * a:32 * a + S, :, 2:2 + T] = xg[:, a].transpose(1, 0, 2)
        ins.append(dict(
            xs=np.ascontiguousarray(xsa.reshape(128, G * PITCH)).astype(BF),
            w1=w1h, w2=w2h, wt=wth, b1=b1h, b2=b2h, bt=bth))
    return ins


def _host_post(F2, adj_self, Wa, Wm1, Wm2, Wm3, Wg1, Wg2, wg,
               Wp1, Wp2, Wp3, Wl, Wgl, Ws1, Ws2, Wf1, Wf2, Wcls, bcls):
    """Numpy port of reference() from F2 onward. F2: [B, S, FD] float32."""
    A = np.asarray(adj_self, np.float32)
    f = lambda w: np.asarray(w, np.float32)
    relu = lambda v: np.maximum(v, 0.0)
    P = F2 @ f(Wa)                                   # [B,S,FD]
    M = np.einsum('big,bjg->bij', P, F2)
    Mr = relu(M)
    E = np.exp(Mr - Mr.max(-1, keepdims=True))
    A_F = E / E.sum(-1, keepdims=True)
    gc = lambda Am, X, W: relu(np.einsum('bij,bjf->bif', Am, X) @ W) \
        if Am.ndim == 3 else relu(np.einsum('ij,bjf->bif', Am, X) @ W)
    x1 = gc(A_F, F2, f(Wm1))
    x2 = gc(A_F, x1, f(Wm2))
    x3 = gc(A_F, x2, f(Wm3))
    h1 = relu(np.einsum('ij,bjf->bif', A, F2) @ f(Wg1))
    xs = np.einsum('ij,bjf->bif', A, h1) @ f(Wg2)
    H1, H2, H3 = (x1 + xs) * .5, (x2 + xs) * .5, (x3 + xs) * .5
    wgf = f(wg)
    sc = np.stack([H @ wgf[:, k] for k, H in enumerate((H1, H2, H3))], -1)
    e = np.exp(sc - sc.max(-1, keepdims=True))
    g = e / e.sum(-1, keepdims=True)
    agg = lambda k, H, Wp: np.einsum('ij,bjf->bif',
                                     A, g[..., k:k + 1] * H) @ f(Wp)
    G_h = np.concatenate([agg(0, H1, Wp1), agg(1, H2, Wp2),
                          agg(2, H3, Wp3)], -1)
    loc = relu(np.einsum('ij,bjf->bif', A, F2) @ f(Wl))
    glb = relu(np.einsum('bij,bjf->bif', A_F, F2) @ f(Wgl))
    G_v = np.concatenate([loc, glb], -1)
    sig = lambda v: 1.0 / (1.0 + np.exp(-v))
    wch = sig(relu(G_v.mean(-1) @ f(Ws1)) @ f(Ws2))
    G_h_att = G_h * wch[:, :, None]
    wft = sig(relu(G_h.mean(1) @ f(Wf1)) @ f(Wf2))
    G_v_att = G_v * wft[:, None, :]
    Gc = np.concatenate([G_h_att, G_v_att], -1).reshape(F2.shape[0], -1)
    logits = Gc @ f(Wcls) + f(bcls)
    lse = logits - logits.max(-1, keepdims=True)
    return (lse - np.log(np.exp(lse).sum(-1, keepdims=True))).astype(np.float32)


def kernel(x, adj_self, Wc1, bc1, Wc2, bc2, Wt, bt, Wa, Wm1, Wm2, Wm3,
           Wg1, Wg2, wg, Wp1, Wp2, Wp3, Wl, Wgl, Ws1, Ws2, Wf1, Wf2,
           Wcls, bcls, _trace=False):
    nc = _get_nc()
    ins = _host_inputs(x, Wc1, bc1, Wc2, bc2, Wt, bt)
    res = bass_utils.run_bass_kernel_spmd(
        nc, ins, core_ids=list(range(NCORES)), trace=_trace)
    F2 = np.empty((B, S, FD), np.float32)
    for c in range(NCORES):
        o = np.asarray(res.results[c]["f2o"], np.float32)  # [FD, 4096]
        o4 = o.reshape(FD, G, 4, 32)                        # f, group, a, slot
        F2[c * BC:(c + 1) * BC] = o4[:, :, :, :S].transpose(1, 2, 3, 0).reshape(
            BC, S, FD)
    out = _host_post(F2, adj_self, Wa, Wm1, Wm2, Wm3, Wg1, Wg2, wg,
                     Wp1, Wp2, Wp3, Wl, Wgl, Ws1, Ws2, Wf1, Wf2, Wcls, bcls)
    if _trace:
        kernel.last_exec_time_ns = res.exec_time_ns
    return out


# revision 5
# speedup vs baseline: 1.0070x; 1.0030x over previous
"""DualGraphSHM kernel: conv1+conv2+F2 on 8 trn2 cores (bass/tile), rest on host.

Device per core (128 samples): x -> conv1 -> relu -> conv2 -> relu ->
xbar-transpose -> F2 = relu(h2 @ Wt + bt).  Layout: 4 samples stacked on
partitions at 32-row pitch ("slots"), time on the free dim with 2-col zero
padding per sample block so SAME-conv offsets never cross samples.
"""
import numpy as np
import ml_dtypes

import concourse.bacc as bacc
import concourse.mybir as mybir
import concourse.tile as tile
from concourse import bass_utils

BF = ml_dtypes.bfloat16
NCORES = 8
B, S, T, FD, NCLS = 1024, 30, 1024, 200, 7
BC = B // NCORES          # 128 samples per core
G = BC // 4               # 32 groups of 4 stacked samples
GH = G // 2               # 16 groups per half (SBUF pressure)
PITCH = 1028              # per-group col pitch: [2 zero][1024 data][2 zero]

_noisy = False


def _build_device_program():
    nc = bacc.Bacc("TRN2", target_bir_lowering=False, debug=False,
                   num_devices=NCORES)
    dt = mybir.dt
    xs = nc.dram_tensor("xs", [128, G * PITCH], dt.bfloat16,
                        kind="ExternalInput").ap()
    w1 = nc.dram_tensor("w1", [128, 3 * 128], dt.bfloat16,
                        kind="ExternalInput").ap()
    w2 = nc.dram_tensor("w2", [128, 3 * 128], dt.bfloat16,
                        kind="ExternalInput").ap()
    wt = nc.dram_tensor("wt", [128, 8 * FD], dt.bfloat16,
                        kind="ExternalInput").ap()
    b1 = nc.dram_tensor("b1", [128, 1], dt.float32, kind="ExternalInput").ap()
    b2 = nc.dram_tensor("b2", [128, 1], dt.float32, kind="ExternalInput").ap()
    bt = nc.dram_tensor("bt", [128, 2], dt.float32, kind="ExternalInput").ap()
    f2o = nc.dram_tensor("f2o", [FD, BC * 32], dt.float32,
                         kind="ExternalOutput").ap()

    with tile.TileContext(nc) as tc:
        with (
            tc.tile_pool(name="consts", bufs=1) as cpool,
            tc.tile_pool(name="big", bufs=1) as bpool,
            tc.tile_pool(name="ps", bufs=6, space="PSUM") as pspool,
            tc.tile_pool(name="f2ps", bufs=2, space="PSUM") as f2pspool,
            tc.tile_pool(name="ev", bufs=2) as evpool,
            tc.tile_pool(name="xin", bufs=2) as xpool,
        ):
            w1s = cpool.tile([128, 3 * 128], dt.bfloat16, tag="w1")
            w2s = cpool.tile([128, 3 * 128], dt.bfloat16, tag="w2")
            wts = cpool.tile([128, 8 * FD], dt.bfloat16, tag="wt")
            b1s = cpool.tile([128, 1], dt.float32, tag="b1")
            b2s = cpool.tile([128, 1], dt.float32, tag="b2")
            bts = cpool.tile([128, 2], dt.float32, tag="bt")
            nc.sync.dma_start(w1s[:], w1[:])
            nc.sync.dma_start(w2s[:], w2[:])
            nc.sync.dma_start(wts[:], wt[:])
            nc.sync.dma_start(b1s[:], b1[:])
            nc.sync.dma_start(b2s[:], b2[:])
            nc.sync.dma_start(bts[:], bt[:])

            relu = mybir.ActivationFunctionType.Relu

            for half in range(2):
                g0 = half * GH
                xsh = xpool.tile([128, GH * PITCH], dt.bfloat16, tag="xsh")
                nc.gpsimd.dma_start(
                    xsh[:], xs[:, g0 * PITCH:(g0 + GH) * PITCH])
                h1 = bpool.tile([128, GH * PITCH], dt.bfloat16, tag="h1")
                # zero the 4 pad cols of each block (conv2 reads them)
                nc.vector.memset(
                    h1[:].rearrange("p (g c) -> p g c", c=PITCH)[:, :, 0:2], 0.0)
                nc.vector.memset(
                    h1[:].rearrange("p (g c) -> p g c", c=PITCH)[:, :, 1026:1028],
                    0.0)
                # ---- conv1 ----
                for g in range(GH):
                    base = g * PITCH
                    for hlf in range(2):
                        ps = pspool.tile([128, 512], dt.float32, tag="cps")
                        for k in range(3):
                            nc.tensor.matmul(
                                ps[:],
                                w1s[:, k * 128:(k + 1) * 128],
                                xsh[:, base + 1 + hlf * 512 + k:
                                    base + 1 + hlf * 512 + k + 512],
                                start=(k == 0), stop=(k == 2))
                        if hlf == 0:
                            nc.scalar.activation(
                                h1[:, base + 2:base + 2 + 512],
                                ps[:], relu, bias=b1s[:])
                        else:
                            nc.vector.tensor_scalar(
                                h1[:, base + 2 + 512:base + 2 + 1024],
                                ps[:], b1s[:], 0.0,
                                mybir.AluOpType.add, mybir.AluOpType.max)
                # ---- conv2 ----
                h2 = bpool.tile([128, GH * 1024], dt.bfloat16, tag="h2")
                for g in range(GH):
                    base = g * PITCH
                    for hlf in range(2):
                        ps = pspool.tile([128, 512], dt.float32, tag="cps")
                        for k in range(3):
                            nc.tensor.matmul(
                                ps[:],
                                w2s[:, k * 128:(k + 1) * 128],
                                h1[:, base + 1 + hlf * 512 + k:
                                   base + 1 + hlf * 512 + k + 512],
                                start=(k == 0), stop=(k == 2))
                        if hlf == 0:
                            nc.scalar.activation(
                                h2[:, g * 1024:g * 1024 + 512],
                                ps[:], relu, bias=b2s[:])
                        else:
                            nc.vector.tensor_scalar(
                                h2[:, g * 1024 + 512:g * 1024 + 1024],
                                ps[:], b2s[:], 0.0,
                                mybir.AluOpType.add, mybir.AluOpType.max)
                # ---- transpose h2: per group [128, 1024] -> [128, 8, 128] ----
                h2t = bpool.tile([128, GH * 1024], dt.bfloat16, tag="h2t")
                for g in range(GH):
                    nc.sync.dma_start(
                        h2t[:, g * 1024:(g + 1) * 1024].rearrange(
                            "p (c f) -> p c f", f=128),
                        h2[:, g * 1024:(g + 1) * 1024],
                        transpose=True)
                # ---- F2^T = relu(Wt^T @ h2t + bt) ----
                for sub in range(4):          # 4 groups per matmul -> N=512
                    for ft in range(2):       # f tiles: 128 + 72
                        fw = 128 if ft == 0 else FD - 128
                        ps = f2pspool.tile([128, 512], dt.float32, tag="f2ps")
                        for tt in range(8):
                            nc.tensor.matmul(
                                ps[:fw, :],
                                wts[:, tt * FD + ft * 128:tt * FD + ft * 128 + fw],
                                h2t[:, :].rearrange(
                                    "p (g c s) -> p g c s", c=8, s=128)[
                                    :, sub * 4:(sub + 1) * 4, tt, :],
                                start=(tt == 0), stop=(tt == 7))
                        ev = evpool.tile([128, 512], dt.float32, tag="f2ev")
                        nc.scalar.activation(ev[:fw, :], ps[:fw, :], relu,
                                             bias=bts[:fw, ft:ft + 1])
                        nc.sync.dma_start(
                            f2o[ft * 128:ft * 128 + fw,
                                half * 2048 + sub * 512:
                                half * 2048 + sub * 512 + 512],
                            ev[:fw, :])
    nc.compile()
    return nc


_nc_cache = None


def _get_nc():
    global _nc_cache
    if _nc_cache is None:
        _nc_cache = _build_device_program()
    return _nc_cache


def _host_inputs(x, Wc1, bc1, Wc2, bc2, Wt, bt):
    """Build per-core input dicts."""
    xf = np.asarray(x, np.float32)
    ins = []
    # block-diag conv weights: lhsT[32a+i, 32a+o] = W[o, i, k]
    def bdw(W):
        out = np.zeros((128, 3 * 128), np.float32)
        for k in range(3):
            for a in range(4):
                out[32 * a:32 * a + S, k * 128 + 32 * a:k * 128 + 32 * a + S] = \
                    np.asarray(W, np.float32)[:, :, k].T
        return out.astype(BF)
    w1h, w2h = bdw(Wc1), bdw(Wc2)
    wth = np.zeros((128, 8 * FD), np.float32)
    wtf = np.asarray(Wt, np.float32)
    for tt in range(8):
        wth[:, tt * FD:(tt + 1) * FD] = wtf[tt * 128:(tt + 1) * 128, :]
    wth = wth.astype(BF)
    bstack = np.zeros((128, 1), np.float32)
    b1h = bstack.copy(); b2h = bstack.copy()
    for a in range(4):
        b1h[32 * a:32 * a + S, 0] = np.asarray(bc1, np.float32)
        b2h[32 * a:32 * a + S, 0] = np.asarray(bc2, np.float32)
    bth = np.zeros((128, 2), np.float32)
    btf = np.asarray(bt, np.float32)
    bth[:, 0] = btf[:128]
    bth[:72, 1] = btf[128:]
    for c in range(NCORES):
        xc = xf[c * BC:(c + 1) * BC]            # [128, 30, 1024]
        xsa = np.zeros((128, G, PITCH), np.float32)
        xg = xc.reshape(G, 4, S, T)             # sample = 4*g + a
        for a in range(4):
            xsa[32 * a:32 * a + S, :, 2:2 + T] = xg[:, a].transpose(1, 0, 2)
        ins.append(dict(
            xs=np.ascontiguousarray(xsa.reshape(128, G * PITCH)).astype(BF),
            w1=w1h, w2=w2h, wt=wth, b1=b1h, b2=b2h, bt=bth))
    return ins


def _host_post(F2, adj_self, Wa, Wm1, Wm2, Wm3, Wg1, Wg2, wg,
               Wp1, Wp2, Wp3, Wl, Wgl, Ws1, Ws2, Wf1, Wf2, Wcls, bcls):
    """Numpy port of reference() from F2 onward. F2: [B, S, FD] float32."""
    A = np.asarray(adj_self, np.float32)
    f = lambda w: np.asarray(w, np.float32)
    relu = lambda v: np.maximum(v, 0.0)
    P = F2 @ f(Wa)                                   # [B,S,FD]
    M = np.einsum('big,bjg->bij', P, F2)
    Mr = relu(M)
    E = np.exp(Mr - Mr.max(-1, keepdims=True))
    A_F = E / E.sum(-1, keepdims=True)
    gc = lambda Am, X, W: relu(np.einsum('bij,bjf->bif', Am, X) @ W) \
        if Am.ndim == 3 else relu(np.einsum('ij,bjf->bif', Am, X) @ W)
    x1 = gc(A_F, F2, f(Wm1))
    x2 = gc(A_F, x1, f(Wm2))
    x3 = gc(A_F, x2, f(Wm3))
    h1 = relu(np.einsum('ij,bjf->bif', A, F2) @ f(Wg1))
    xs = np.einsum('ij,bjf->bif', A, h1) @ f(Wg2)
    H1, H2, H3 = (x1 + xs) * .5, (x2 + xs) * .5, (x3 + xs) * .5
    wgf = f(wg)
    sc = np.stack([H @ wgf[:, k] for k, H in enumerate((H1, H2, H3))], -1)
    e = np.exp(sc - sc.max(-1, keepdims=True))
    g = e / e.sum(-1, keepdims=True)
    agg = lambda k, H, Wp: np.einsum('ij,bjf->bif',
                                     A, g[..., k:k + 1] * H) @ f(Wp)
    G_h = np.concatenate([agg(0, H1, Wp1), agg(1, H2, Wp2),
                          agg(2, H3, Wp3)], -1)
    loc = relu(np.einsum('ij,bjf->bif', A, F2) @ f(Wl))
    glb = relu(np.einsum('bij,bjf->bif', A_F, F2) @ f(Wgl))
    G_v = np.concatenate([loc, glb], -1)
    sig = lambda v: 1.0 / (1.0 + np.exp(-v))
    wch = sig(relu(G_v.mean(-1) @ f(Ws1)) @ f(Ws2))
    G_h_att = G_h * wch[:, :, None]
    wft = sig(relu(G_h.mean(1) @ f(Wf1)) @ f(Wf2))
    G_v_att = G_v * wft[:, None, :]
    Gc = np.concatenate([G_h_att, G_v_att], -1).reshape(F2.shape[0], -1)
    logits = Gc @ f(Wcls) + f(bcls)
    lse = logits - logits.max(-1, keepdims=True)
    return (lse - np.log(np.exp(lse).sum(-1, keepdims=True))).astype(np.float32)


def kernel(x, adj_self, Wc1, bc1, Wc2, bc2, Wt, bt, Wa, Wm1, Wm2, Wm3,
           Wg1, Wg2, wg, Wp1, Wp2, Wp3, Wl, Wgl, Ws1, Ws2, Wf1, Wf2,
           Wcls, bcls, _trace=False):
    nc = _get_nc()
    ins = _host_inputs(x, Wc1, bc1, Wc2, bc2, Wt, bt)
    res = bass_utils.run_bass_kernel_spmd(
        nc, ins, core_ids=list(range(NCORES)), trace=_trace)
    F2 = np.empty((B, S, FD), np.float32)
    for c in range(NCORES):
        o = np.asarray(res.results[c]["f2o"], np.float32)  # [FD, 4096]
        o4 = o.reshape(FD, G, 4, 32)                        # f, group, a, slot
        F2[c * BC:(c + 1) * BC] = o4[:, :, :, :S].transpose(1, 2, 3, 0).reshape(
            BC, S, FD)
    out = _host_post(F2, adj_self, Wa, Wm1, Wm2, Wm3, Wg1, Wg2, wg,
                     Wp1, Wp2, Wp3, Wl, Wgl, Ws1, Ws2, Wf1, Wf2, Wcls, bcls)
    if _trace:
        kernel.last_exec_time_ns = res.exec_time_ns
    return out
